# revision 1
# baseline (speedup 1.0000x reference)
"""HAN layer (4 metapaths x 2-layer mean-RGCN + metapath attention) on 8 trn2 cores.

Sharding: cores (2i, 2i+1) handle metapath i. Within a pair, L1 splits dst into
halves [0,nreg)/[nreg,2*nreg); after an in-pair AllGather of x1, L2 splits the
NREG range into quarters. Attention: score AllGather + ReduceScatter over the 4
cores holding the same node range ({0,2,4,6} and {1,3,5,7}).

Device algorithm per layer (linearity: segment_sum(x[src]) @ Wm): edges are
host-sorted by dst into groups of 128 dsts; an indirect DMA gathers x[src] rows
for a group; per 128-edge chunk a selector eq[e,d] = (dl[e]==d)*rec[e] is built
on DVE and matmul-accumulated on PE into meanT = (segment_mean)^T in PSUM; two
dense matmuls + fused ReLU produce the group's 128 output rows, written
contiguously (no scatter anywhere).
"""

import math
import numpy as np

import concourse.bass as bass
import concourse.bacc as bacc
import concourse.mybir as mybir
from concourse.tile import TileContext
from concourse.bass_utils import run_bass_kernel_spmd

F32 = mybir.dt.float32
I32 = mybir.dt.int32

N_CORES = 8
BF = 4     # output groups batched per store DMA
CH = 16    # groups per grid-load DMA


# ----------------------------------------------------------------- host prep

def _build_grids(srcs, dsts, lo, ng, nb, rec):
    """grid[p, g*nb + b] = edge at (partition p, chunk b) of group g; the
    indirect-DMA flat order j = p*nb + b lands row j at out-partition p,
    column block b. Empty slots: dl=128 (selector row all-zero)."""
    g = (dsts - lo) >> 7
    starts = np.searchsorted(dsts, lo + 128 * np.arange(ng))
    slot = np.arange(len(dsts)) - starts[g]
    p = slot & 127
    b = slot >> 7
    col = g * nb + b
    idx_g = np.zeros((128, nb * ng), np.int32)
    dl_g = np.full((128, nb * ng), 128.0, np.float32)
    rec_g = np.zeros((128, nb * ng), np.float32)
    idx_g[p, col] = srcs
    dl_g[p, col] = (dsts - lo - (g << 7)).astype(np.float32)
    rec_g[p, col] = rec[dsts]
    return idx_g, dl_g, rec_g


def _group_max(dsts, lo, ng):
    starts = np.searchsorted(dsts, lo + 128 * np.arange(ng + 1))
    return int(np.diff(starts).max()) if len(dsts) else 1


# ------------------------------------------------------------- device build

def _emit_layer(nc, tc, pools, table, gidx, gdl, grec, gidxd, wm_t, wr_t,
                ng, nb, iota_t, ident_t, out_dram, rows_total, hook=None):
    sb, sbg, psum, sbeq = pools
    nch = math.ceil(ng / CH)
    stage = None
    for g in range(ng):
        if g % CH == 0:
            w = min(CH, ng - g)
            idxt = sbg.tile([128, nb * w], I32, tag="idxt")
            nc.sync.dma_start(out=idxt[:], in_=gidx[:, g * nb:(g + w) * nb])
            dlt = sbg.tile([128, nb * w], F32, tag="dlt")
            nc.sync.dma_start(out=dlt[:], in_=gdl[:, g * nb:(g + w) * nb])
            rect = sbg.tile([128, nb * w], F32, tag="rect")
            nc.sync.dma_start(out=rect[:], in_=grec[:, g * nb:(g + w) * nb])
            idxdt = sbg.tile([128, w], I32, tag="idxdt")
            nc.sync.dma_start(out=idxdt[:], in_=gidxd[:, g:g + w])
        o = (g % CH) * nb

        msgs = sb.tile([128, nb * 128], F32, tag="msgs")
        for b in range(nb):
            nc.gpsimd.indirect_dma_start(
                out=msgs[:, b * 128:(b + 1) * 128], out_offset=None,
                in_=table[:],
                in_offset=bass.IndirectOffsetOnAxis(
                    ap=idxt[:, o + b:o + b + 1], axis=0))

        meant_ps = psum.tile([128, 128], F32, space="PSUM", tag="meant")
        for b in range(nb):
            eq = sbeq.tile([128, 128], F32, tag="eq")
            nc.vector.tensor_scalar(
                out=eq[:], in0=iota_t[:],
                scalar1=dlt[:, o + b:o + b + 1], scalar2=rect[:, o + b:o + b + 1],
                op0=mybir.AluOpType.is_equal, op1=mybir.AluOpType.mult)
            nc.tensor.matmul(out=meant_ps[:], lhsT=msgs[:, b * 128:(b + 1) * 128],
                             rhs=eq[:], start=(b == 0), stop=(b == nb - 1))
        meant = sb.tile([128, 128], F32, tag="meant_sb")
        nc.vector.tensor_copy(out=meant[:], in_=meant_ps[:])

        xd = sb.tile([128, 128], F32, tag="xd")
        nc.gpsimd.indirect_dma_start(
            out=xd[:], out_offset=None, in_=table[:],
            in_offset=bass.IndirectOffsetOnAxis(
                ap=idxdt[:, g % CH:g % CH + 1], axis=0))
        xdt_ps = psum.tile([128, 128], F32, space="PSUM", tag="xdt")
        nc.tensor.transpose(out=xdt_ps[:], in_=xd[:], identity=ident_t[:])
        xdt = sb.tile([128, 128], F32, tag="xdt_sb")
        nc.vector.tensor_copy(out=xdt[:], in_=xdt_ps[:])

        h_ps = psum.tile([128, 128], F32, space="PSUM", tag="hps")
        nc.tensor.matmul(out=h_ps[:], lhsT=meant[:], rhs=wm_t[:],
                         start=True, stop=False)
        nc.tensor.matmul(out=h_ps[:], lhsT=xdt[:], rhs=wr_t[:],
                         start=False, stop=True)

        gb = g % BF
        if gb == 0:
            bw = min(BF, ng - g)
            stage = sb.tile([128, bw * 128], F32, tag="xn_stage")
        xn = stage[:, gb * 128:(gb + 1) * 128]
        nc.scalar.activation(out=xn, in_=h_ps[:],
                             func=mybir.ActivationFunctionType.Relu)
        if hook is not None:
            hook(g, xn)
        if gb == bw - 1:
            g0 = g - gb
            rows = min((gb + 1) * 128, rows_total - g0 * 128)
            nfull = rows // 128
            if nfull > 0:
                nc.sync.dma_start(
                    out=out_dram[g0 * 128:g0 * 128 + nfull * 128, :]
                    .rearrange("(a t) f -> t a f", t=128),
                    in_=stage[:, :nfull * 128]
                    .rearrange("p (a f) -> p a f", f=128))
            rem = rows - nfull * 128
            if rem > 0:
                nc.sync.dma_start(
                    out=out_dram[g0 * 128 + nfull * 128:
                                 g0 * 128 + nfull * 128 + rem, :],
                    in_=stage[:rem, nfull * 128:(nfull + 1) * 128])


def build_program(n, nreg, ng1, nb1, ng2, nb2, debug=False):
    nc = bacc.Bacc("TRN2", target_bir_lowering=False, debug=False,
                   num_devices=N_CORES)
    half = nreg
    nrs = (ng2 * 128) // 4  # ReduceScatter rows per rank

    ei = lambda name, shape, dt=F32: nc.dram_tensor(name, shape, dt,
                                                    kind="ExternalInput")
    x0 = ei("x0", [n, 128])
    g1_idx = ei("g1_idx", [128, nb1 * ng1], I32)
    g1_dl = ei("g1_dl", [128, nb1 * ng1])
    g1_rec = ei("g1_rec", [128, nb1 * ng1])
    g1_idxd = ei("g1_idxd", [128, ng1], I32)
    g2_idx = ei("g2_idx", [128, nb2 * ng2], I32)
    g2_dl = ei("g2_dl", [128, nb2 * ng2])
    g2_rec = ei("g2_rec", [128, nb2 * ng2])
    g2_idxd = ei("g2_idxd", [128, ng2], I32)
    wm1, wr1 = ei("wm1", [128, 128]), ei("wr1", [128, 128])
    wm2, wr2 = ei("wm2", [128, 128]), ei("wr2", [128, 128])
    qs_rep = ei("qs_rep", [128, 128])
    sel = ei("sel", [128, 4])
    iota_in = ei("iota", [128, 128])
    ident_in = ei("ident", [128, 128])

    out_part = nc.dram_tensor("out_part", [nrs, 128], F32,
                              kind="ExternalOutput")

    x1_half = nc.dram_tensor("x1_half", [half, 128], F32)
    x1_full = nc.dram_tensor("x1_full", [n, 128], F32)
    x2b = nc.dram_tensor("x2b", [ng2 * 128, 128], F32)
    sc_in = nc.dram_tensor("sc_in", [ng2, 128], F32)
    sc_all = nc.dram_tensor("sc_all", [4 * ng2, 128], F32)
    rs_in = nc.dram_tensor("rs_in", [ng2 * 128, 128], F32)
    rs_out = nc.dram_tensor("rs_out", [nrs, 128], F32)

    pair_groups = [[2 * i, 2 * i + 1] for i in range(4)]
    attn_groups = [[0, 2, 4, 6], [1, 3, 5, 7]]

    with TileContext(nc) as tc:
        with (
            tc.tile_pool(name="const", bufs=1) as cpool,
            tc.tile_pool(name="sb", bufs=3) as sb,
            tc.tile_pool(name="sbg", bufs=2) as sbg,
            tc.tile_pool(name="sbeq", bufs=4) as sbeq,
            tc.tile_pool(name="psum", bufs=2, space="PSUM") as psum,
        ):
            def cload(src, shape, tag):
                t = cpool.tile(shape, F32, tag=tag)
                nc.sync.dma_start(out=t[:], in_=src[:, :])
                return t

            iota_t = cload(iota_in, [128, 128], "c_iota")
            ident_t = cload(ident_in, [128, 128], "c_ident")
            wm1_t = cload(wm1, [128, 128], "c_wm1")
            wr1_t = cload(wr1, [128, 128], "c_wr1")
            wm2_t = cload(wm2, [128, 128], "c_wm2")
            wr2_t = cload(wr2, [128, 128], "c_wr2")
            qs_t = cload(qs_rep, [128, 128], "c_qs")
            sel_t = cload(sel, [128, 4], "c_sel")
            score_sb = cpool.tile([128, ng2], F32, tag="c_score")

            pools = (sb, sbg, psum, sbeq)

            _emit_layer(nc, tc, pools, x0, g1_idx, g1_dl, g1_rec, g1_idxd,
                        wm1_t, wr1_t, ng1, nb1, iota_t, ident_t,
                        x1_half, half)

            nc.gpsimd.collective_compute(
                "AllGather", mybir.AluOpType.bypass,
                replica_groups=pair_groups,
                ins=[x1_half[:, :]], outs=[x1_full[:, :]])

            def score_hook(g, xn):
                t = sb.tile([128, 128], F32, tag="sc_tmp")
                nc.vector.tensor_tensor(out=t[:], in0=xn, in1=qs_t[:],
                                        op=mybir.AluOpType.mult)
                nc.vector.reduce_sum(out=score_sb[:, g:g + 1], in_=t[:],
                                     axis=mybir.AxisListType.X)

            _emit_layer(nc, tc, pools, x1_full, g2_idx, g2_dl, g2_rec, g2_idxd,
                        wm2_t, wr2_t, ng2, nb2, iota_t, ident_t,
                        x2b, ng2 * 128, hook=score_hook)

            nc.sync.dma_start(out=sc_in[:, :].rearrange("t p -> p t"),
                              in_=score_sb[:, :])
            nc.gpsimd.collective_compute(
                "AllGather", mybir.AluOpType.bypass,
                replica_groups=attn_groups,
                ins=[sc_in[:, :]], outs=[sc_all[:, :]])

            # softmax over 4 metapaths (elementwise across four [128,ng2] tiles)
            s_t = []
            for p in range(4):
                st = cpool.tile([128, ng2], F32, tag=f"s{p}")
                nc.sync.dma_start(
                    out=st[:],
                    in_=sc_all[p * ng2:(p + 1) * ng2, :].rearrange("t p -> p t"))
                s_t.append(st)
            m = cpool.tile([128, ng2], F32, tag="c_m")
            nc.vector.tensor_tensor(out=m[:], in0=s_t[0][:], in1=s_t[1][:],
                                    op=mybir.AluOpType.max)
            for p in (2, 3):
                nc.vector.tensor_tensor(out=m[:], in0=m[:], in1=s_t[p][:],
                                        op=mybir.AluOpType.max)
            e_t = []
            for p in range(4):
                dt_ = cpool.tile([128, ng2], F32, tag=f"d{p}")
                nc.vector.tensor_tensor(out=dt_[:], in0=s_t[p][:], in1=m[:],
                                        op=mybir.AluOpType.subtract)
                et = cpool.tile([128, ng2], F32, tag=f"e{p}")
                nc.scalar.activation(out=et[:], in_=dt_[:],
                                     func=mybir.ActivationFunctionType.Exp)
                e_t.append(et)
            z = cpool.tile([128, ng2], F32, tag="c_z")
            nc.vector.tensor_tensor(out=z[:], in0=e_t[0][:], in1=e_t[1][:],
                                    op=mybir.AluOpType.add)
            for p in (2, 3):
                nc.vector.tensor_tensor(out=z[:], in0=z[:], in1=e_t[p][:],
                                        op=mybir.AluOpType.add)
            rz = cpool.tile([128, ng2], F32, tag="c_rz")
            nc.vector.reciprocal(out=rz[:], in_=z[:])
            wown = cpool.tile([128, ng2], F32, tag="c_wown")
            acc = cpool.tile([128, ng2], F32, tag="c_acc")
            nc.vector.tensor_scalar(out=wown[:], in0=e_t[0][:],
                                    scalar1=sel_t[:, 0:1], scalar2=None,
                                    op0=mybir.AluOpType.mult)
            for p in (1, 2, 3):
                nc.vector.tensor_scalar(out=acc[:], in0=e_t[p][:],
                                        scalar1=sel_t[:, p:p + 1], scalar2=None,
                                        op0=mybir.AluOpType.mult)
                nc.vector.tensor_tensor(out=wown[:], in0=wown[:], in1=acc[:],
                                        op=mybir.AluOpType.add)
            nc.vector.tensor_tensor(out=wown[:], in0=wown[:], in1=rz[:],
                                    op=mybir.AluOpType.mult)

            # weighted partials, batched BF groups per DMA
            for g0 in range(0, ng2, BF):
                bw = min(BF, ng2 - g0)
                xt = sb.tile([128, bw * 128], F32, tag="attn_x")
                nc.sync.dma_start(
                    out=xt[:].rearrange("p (a f) -> p a f", f=128),
                    in_=x2b[g0 * 128:(g0 + bw) * 128, :]
                    .rearrange("(a t) f -> t a f", t=128))
                wt = sb.tile([128, bw * 128], F32, tag="attn_w")
                for j in range(bw):
                    nc.vector.tensor_scalar(
                        out=wt[:, j * 128:(j + 1) * 128],
                        in0=xt[:, j * 128:(j + 1) * 128],
                        scalar1=wown[:, g0 + j:g0 + j + 1], scalar2=None,
                        op0=mybir.AluOpType.mult)
                nc.sync.dma_start(
                    out=rs_in[g0 * 128:(g0 + bw) * 128, :]
                    .rearrange("(a t) f -> t a f", t=128),
                    in_=wt[:].rearrange("p (a f) -> p a f", f=128))

            nc.gpsimd.collective_compute(
                "ReduceScatter", mybir.AluOpType.add,
                replica_groups=attn_groups,
                ins=[rs_in[:, :]], outs=[rs_out[:, :]])

            # rs_out [nrs,128] -> out_part, bounced through SBUF
            nblk = nrs // 128
            fin = cpool.tile([128, nblk * 128], F32, tag="c_fin")
            nc.sync.dma_start(
                out=fin[:].rearrange("p (a f) -> p a f", f=128),
                in_=rs_out[:, :].rearrange("(a t) f -> t a f", t=128))
            nc.sync.dma_start(
                out=out_part[:, :].rearrange("(a t) f -> t a f", t=128),
                in_=fin[:].rearrange("p (a f) -> p a f", f=128))

            if debug:
                def dump(src, dst, rows):
                    for r0 in range(0, rows, 128):
                        r = min(128, rows - r0)
                        t = sb.tile([128, 128], F32, tag="dbg")
                        nc.sync.dma_start(out=t[:r, :], in_=src[r0:r0 + r, :])
                        nc.sync.dma_start(out=dst[r0:r0 + r, :], in_=t[:r, :])
                dbg_x1 = nc.dram_tensor("dbg_x1", [n, 128], F32,
                                        kind="ExternalOutput")
                dump(x1_full, dbg_x1, n)
                dbg_x2 = nc.dram_tensor("dbg_x2", [ng2 * 128, 128], F32,
                                        kind="ExternalOutput")
                dump(x2b, dbg_x2, ng2 * 128)
                dbg_sc = nc.dram_tensor("dbg_sc", [4 * ng2, 128], F32,
                                        kind="ExternalOutput")
                dump(sc_all, dbg_sc, 4 * ng2)
                dbg_w = nc.dram_tensor("dbg_w", [128, ng2], F32,
                                       kind="ExternalOutput")
                wt_ = sb.tile([128, ng2], F32, tag="dbg_w")
                nc.vector.tensor_copy(out=wt_[:], in_=wown[:])
                nc.sync.dma_start(out=dbg_w[:, :], in_=wt_[:])
    return nc


# ----------------------------------------------------------------- kernel()

def kernel(E, metapath_emb, W_root, W_rel, b, Wq, bq, edge_index, eids,
           nreg=50000, trace=False, debug=False):
    P = edge_index.shape[0]
    n = eids.shape[1]
    d = E.shape[1]
    scale = np.float32(1.0 / math.sqrt(d))
    assert P == 4 and d == 128 and n == 2 * nreg and nreg % 4 == 0
    assert not np.any(np.asarray(b)), "nonzero bias not supported"

    E = np.asarray(E, np.float32)
    edge_index = np.asarray(edge_index)
    eids = np.asarray(eids)

    query = (np.asarray(metapath_emb, np.float32) @ np.asarray(Wq, np.float32)
             + np.asarray(bq, np.float32))
    query_scaled = query * scale

    ng1 = math.ceil(nreg / 128)
    ng2 = math.ceil((nreg // 2) / 128)

    # per-metapath: x0, degree recip, dst-sorted edges
    metas = []
    for i in range(P):
        src = edge_index[i, 0].astype(np.int32)
        dst = edge_index[i, 1].astype(np.int32)
        x0 = np.ascontiguousarray(E[eids[i]]).astype(np.float32)
        deg = np.bincount(dst, minlength=n).astype(np.float32)
        rec = (1.0 / np.maximum(deg, 1.0)).astype(np.float32)
        order = np.argsort(dst, kind="stable")
        metas.append((x0, rec, src[order], dst[order]))

    def rng(i, lo, hi):
        _, _, ssrc, sdst = metas[i]
        a, bb = np.searchsorted(sdst, [lo, hi])
        return ssrc[a:bb], sdst[a:bb]

    spans = []
    for c in range(N_CORES):
        i, h = c // 2, c % 2
        lo1, lo2 = h * nreg, h * (nreg // 2)
        spans.append((rng(i, lo1, lo1 + ng1 * 128),
                      rng(i, lo2, lo2 + ng2 * 128), lo1, lo2))

    nb1 = max(1, max(math.ceil(_group_max(s[0][1], s[2], ng1) / 128)
                     for s in spans))
    nb2 = max(1, max(math.ceil(_group_max(s[1][1], s[3], ng2) / 128)
                     for s in spans))

    iota = np.tile(np.arange(128, dtype=np.float32), (128, 1))
    ident = np.eye(128, dtype=np.float32)

    in_maps = []
    for c in range(N_CORES):
        i, h = c // 2, c % 2
        (s1, d1), (s2, d2), lo1, lo2 = spans[c]
        rec = metas[i][1]
        i1, l1, r1 = _build_grids(s1, d1, lo1, ng1, nb1, rec)
        i2, l2, r2 = _build_grids(s2, d2, lo2, ng2, nb2, rec)
        idxd1 = np.minimum(lo1 + 128 * np.arange(ng1)[None, :]
                           + np.arange(128)[:, None], n - 1).astype(np.int32)
        idxd2 = np.minimum(lo2 + 128 * np.arange(ng2)[None, :]
                           + np.arange(128)[:, None], n - 1).astype(np.int32)
        selm = np.zeros((128, 4), np.float32)
        selm[:, i] = 1.0
        in_maps.append(dict(
            x0=metas[i][0], g1_idx=i1, g1_dl=l1, g1_rec=r1,
            g1_idxd=np.ascontiguousarray(idxd1),
            g2_idx=i2, g2_dl=l2, g2_rec=r2,
            g2_idxd=np.ascontiguousarray(idxd2),
            wm1=np.ascontiguousarray(W_rel[i, 0]).astype(np.float32),
            wr1=np.ascontiguousarray(W_root[i, 0]).astype(np.float32),
            wm2=np.ascontiguousarray(W_rel[i, 1]).astype(np.float32),
            wr2=np.ascontiguousarray(W_root[i, 1]).astype(np.float32),
            qs_rep=np.tile(query_scaled[i], (128, 1)).astype(np.float32),
            sel=selm, iota=iota, ident=ident,
        ))

    nc = build_program(n, nreg, ng1, nb1, ng2, nb2, debug=debug)
    nc.compile()
    kernel.last_nc = nc
    kernel.last_in_maps = in_maps
    res = run_bass_kernel_spmd(nc, in_maps, core_ids=list(range(N_CORES)),
                               trace=trace)

    q = nreg // 2
    a_rows = np.concatenate([res.results[c]["out_part"] for c in (0, 2, 4, 6)],
                            axis=0)[:q]
    b_rows = np.concatenate([res.results[c]["out_part"] for c in (1, 3, 5, 7)],
                            axis=0)[:q]
    out = np.concatenate([a_rows, b_rows], axis=0).astype(np.float32)
    kernel.last_results = res
    return out



# revision 7
# speedup vs baseline: 2.0207x; 2.0207x over previous
"""HAN layer (4 metapaths x 2-layer mean-RGCN + metapath attention) on 8 trn2 cores.

Sharding: cores (2i, 2i+1) handle metapath i. Within a pair, L1 splits dst into
halves [0,nreg)/[nreg,2*nreg); after an in-pair AllGather of x1, L2 splits the
NREG range into quarters. Attention: score AllGather + ReduceScatter over the 4
cores holding the same node range ({0,2,4,6} and {1,3,5,7}).

Device algorithm per layer (linearity: segment_sum(x[src]) @ Wm): edges are
host-sorted by dst into groups of 128 dsts; an indirect DMA gathers x[src] rows
for a group; per 128-edge chunk a selector eq[e,d] = (dl[e]==d)*rec[e] is built
on DVE and matmul-accumulated on PE into meanT = (segment_mean)^T in PSUM; two
dense matmuls + fused ReLU produce the group's 128 output rows, written
contiguously (no scatter anywhere).

Host->device transfer is the end-to-end bottleneck (narrow tunnel link), so
inputs are shipped compressed: E in bf16 sharded 1/8 per core (device AllGather
rebuilds the full table; layer-1 gather indices are composed as eids[src] so
the per-metapath x0 never ships), edge grids at 5 bytes/slot (u16 idx-lo +
u8 idx-hi + u8 deg + u8 dst-local), unpacked on the DVE. All device compute is
bf16 with f32 PSUM accumulation.
"""

import math
import numpy as np

import concourse.bass as bass
import concourse.bacc as bacc
import concourse.mybir as mybir
from concourse.tile import TileContext
from concourse.bass_utils import run_bass_kernel_spmd

F32 = mybir.dt.float32
BF16 = mybir.dt.bfloat16
I32 = mybir.dt.int32
U16 = mybir.dt.uint16
U8 = mybir.dt.uint8

N_CORES = 8
BF = 4     # output groups batched per store DMA
CH = 16    # groups per grid-load DMA


def _np_bf16():
    import ml_dtypes
    return ml_dtypes.bfloat16


# ----------------------------------------------------------------- host prep

def _build_grids(idxs, dsts, lo, ng, nb, deg):
    """Packed grid: grid[p, g*nb + b] = edge at (partition p, chunk b) of group
    g; the indirect-DMA flat order j = p*nb + b lands row j at out-partition p,
    column block b. Ships as u16 idx-low + u8 idx-high + u8 deg + u8 dst-local.
    Empty slots: dl=128 (selector row all-zero), deg=1 (finite reciprocal)."""
    g = (dsts - lo) >> 7
    starts = np.searchsorted(dsts, lo + 128 * np.arange(ng))
    slot = np.arange(len(dsts)) - starts[g]
    p = slot & 127
    b = slot >> 7
    col = g * nb + b
    S = nb * ng
    lo16 = np.zeros((128, S), np.uint16)
    hi8 = np.zeros((128, S), np.uint8)
    dl8 = np.full((128, S), 128, np.uint8)
    dg8 = np.ones((128, S), np.uint8)
    lo16[p, col] = (idxs & 0xFFFF).astype(np.uint16)
    hi8[p, col] = (idxs >> 16).astype(np.uint8)
    dl8[p, col] = (dsts - lo - (g << 7)).astype(np.uint8)
    dg8[p, col] = deg[dsts].astype(np.uint8)
    return lo16, hi8, dl8, dg8


def _group_max(dsts, lo, ng):
    starts = np.searchsorted(dsts, lo + 128 * np.arange(ng + 1))
    return int(np.diff(starts).max()) if len(dsts) else 1


# ------------------------------------------------------------- device build

def _emit_layer(nc, tc, pools, table, glo, ghi, gdl, gdg, gidxd, wm_t, wr_t,
                ng, nb, iota_t, ident_t, out_dram, rows_total, hook=None):
    sb, sbg, psum, sbeq = pools
    stage = None
    for g in range(ng):
        if g % CH == 0:
            w = min(CH, ng - g)
            lot = sbg.tile([128, nb * w], U16, tag="lot")
            nc.sync.dma_start(out=lot[:], in_=glo[:, g * nb:(g + w) * nb])
            hit = sbg.tile([128, nb * w], U8, tag="hit")
            nc.sync.dma_start(out=hit[:], in_=ghi[:, g * nb:(g + w) * nb])
            dlt8 = sbg.tile([128, nb * w], U8, tag="dlt8")
            nc.sync.dma_start(out=dlt8[:], in_=gdl[:, g * nb:(g + w) * nb])
            dgt8 = sbg.tile([128, nb * w], U8, tag="dgt8")
            nc.sync.dma_start(out=dgt8[:], in_=gdg[:, g * nb:(g + w) * nb])
            idxdt = sbg.tile([128, w], I32, tag="idxdt")
            nc.sync.dma_start(out=idxdt[:], in_=gidxd[:, g:g + w])
            # unpack: idx = lo + 65536*hi (exact in f32: < 2^24), rec = 1/deg
            lof = sbg.tile([128, nb * w], F32, tag="lof")
            nc.vector.tensor_copy(out=lof[:], in_=lot[:])
            hif = sbg.tile([128, nb * w], F32, tag="hif")
            nc.vector.tensor_scalar(out=hif[:], in0=hit[:], scalar1=65536.0,
                                    scalar2=None, op0=mybir.AluOpType.mult)
            idxt = sbg.tile([128, nb * w], I32, tag="idxt")
            nc.vector.tensor_tensor(out=idxt[:], in0=lof[:], in1=hif[:],
                                    op=mybir.AluOpType.add)
            dlf = sbg.tile([128, nb * w], F32, tag="dlf")
            nc.vector.tensor_copy(out=dlf[:], in_=dlt8[:])
            dgf = sbg.tile([128, nb * w], F32, tag="dgf")
            nc.vector.tensor_copy(out=dgf[:], in_=dgt8[:])
            recf = sbg.tile([128, nb * w], F32, tag="recf")
            nc.vector.reciprocal(out=recf[:], in_=dgf[:])
        o = (g % CH) * nb

        msgs = sb.tile([128, nb * 128], BF16, tag="msgs")
        for b in range(nb):
            nc.gpsimd.indirect_dma_start(
                out=msgs[:, b * 128:(b + 1) * 128], out_offset=None,
                in_=table[:],
                in_offset=bass.IndirectOffsetOnAxis(
                    ap=idxt[:, o + b:o + b + 1], axis=0))

        meant_ps = psum.tile([128, 128], F32, space="PSUM", tag="meant")
        for b in range(nb):
            eq = sbeq.tile([128, 128], BF16, tag="eq")
            nc.vector.tensor_scalar(
                out=eq[:], in0=iota_t[:],
                scalar1=dlf[:, o + b:o + b + 1], scalar2=recf[:, o + b:o + b + 1],
                op0=mybir.AluOpType.is_equal, op1=mybir.AluOpType.mult)
            nc.tensor.matmul(out=meant_ps[:], lhsT=msgs[:, b * 128:(b + 1) * 128],
                             rhs=eq[:], start=(b == 0), stop=(b == nb - 1))
        meant = sb.tile([128, 128], BF16, tag="meant_sb")
        nc.vector.tensor_copy(out=meant[:], in_=meant_ps[:])

        xd = sb.tile([128, 128], BF16, tag="xd")
        nc.gpsimd.indirect_dma_start(
            out=xd[:], out_offset=None, in_=table[:],
            in_offset=bass.IndirectOffsetOnAxis(
                ap=idxdt[:, g % CH:g % CH + 1], axis=0))
        xdt_ps = psum.tile([128, 128], BF16, space="PSUM", tag="xdt")
        nc.tensor.transpose(out=xdt_ps[:], in_=xd[:], identity=ident_t[:])
        xdt = sb.tile([128, 128], BF16, tag="xdt_sb")
        nc.vector.tensor_copy(out=xdt[:], in_=xdt_ps[:])

        h_ps = psum.tile([128, 128], F32, space="PSUM", tag="hps")
        nc.tensor.matmul(out=h_ps[:], lhsT=meant[:], rhs=wm_t[:],
                         start=True, stop=False)
        nc.tensor.matmul(out=h_ps[:], lhsT=xdt[:], rhs=wr_t[:],
                         start=False, stop=True)

        gb = g % BF
        if gb == 0:
            bw = min(BF, ng - g)
            stage = sb.tile([128, bw * 128], BF16, tag="xn_stage")
        xn = stage[:, gb * 128:(gb + 1) * 128]
        nc.scalar.activation(out=xn, in_=h_ps[:],
                             func=mybir.ActivationFunctionType.Relu)
        if hook is not None:
            hook(g, xn)
        if gb == bw - 1:
            g0 = g - gb
            rows = min((gb + 1) * 128, rows_total - g0 * 128)
            nfull = rows // 128
            if nfull > 0:
                nc.sync.dma_start(
                    out=out_dram[g0 * 128:g0 * 128 + nfull * 128, :]
                    .rearrange("(a t) f -> t a f", t=128),
                    in_=stage[:, :nfull * 128]
                    .rearrange("p (a f) -> p a f", f=128))
            rem = rows - nfull * 128
            if rem > 0:
                nc.sync.dma_start(
                    out=out_dram[g0 * 128 + nfull * 128:
                                 g0 * 128 + nfull * 128 + rem, :],
                    in_=stage[:rem, nfull * 128:(nfull + 1) * 128])


def build_program(n, nreg, etab, ng1, nb1, ng2, nb2):
    nc = bacc.Bacc("TRN2", target_bir_lowering=False, debug=False,
                   num_devices=N_CORES)
    half = nreg
    nsh = etab // N_CORES
    nrs = (ng2 * 128) // 4  # ReduceScatter rows per rank

    def ei(name, shape, dt=F32):
        return nc.dram_tensor(name, shape, dt, kind="ExternalInput")

    e_sh = ei("e_sh", [nsh, 128], BF16)
    g1_lo = ei("g1_lo", [128, nb1 * ng1], U16)
    g1_hi = ei("g1_hi", [128, nb1 * ng1], U8)
    g1_dl = ei("g1_dl", [128, nb1 * ng1], U8)
    g1_dg = ei("g1_dg", [128, nb1 * ng1], U8)
    g1_idxd = ei("g1_idxd", [128, ng1], I32)
    g2_lo = ei("g2_lo", [128, nb2 * ng2], U16)
    g2_hi = ei("g2_hi", [128, nb2 * ng2], U8)
    g2_dl = ei("g2_dl", [128, nb2 * ng2], U8)
    g2_dg = ei("g2_dg", [128, nb2 * ng2], U8)
    g2_idxd = ei("g2_idxd", [128, ng2], I32)
    wm1, wr1 = ei("wm1", [128, 128], BF16), ei("wr1", [128, 128], BF16)
    wm2, wr2 = ei("wm2", [128, 128], BF16), ei("wr2", [128, 128], BF16)
    qs_rep = ei("qs_rep", [128, 128], BF16)
    sel = ei("sel", [128, 4])
    iota_in = ei("iota", [128, 128])
    ident_in = ei("ident", [128, 128], BF16)

    out_part = nc.dram_tensor("out_part", [nrs, 128], BF16,
                              kind="ExternalOutput")

    e_loc = nc.dram_tensor("e_loc", [nsh, 128], BF16)
    e_full = nc.dram_tensor("e_full", [nsh * N_CORES, 128], BF16)
    x1_half = nc.dram_tensor("x1_half", [half, 128], BF16)
    x1_full = nc.dram_tensor("x1_full", [n, 128], BF16)
    x2b = nc.dram_tensor("x2b", [ng2 * 128, 128], BF16)
    sc_in = nc.dram_tensor("sc_in", [ng2, 128], F32)
    sc_all = nc.dram_tensor("sc_all", [4 * ng2, 128], F32)
    rs_in = nc.dram_tensor("rs_in", [ng2 * 128, 128], BF16)
    rs_out = nc.dram_tensor("rs_out", [nrs, 128], BF16)

    all_group = [list(range(N_CORES))]
    pair_groups = [[2 * i, 2 * i + 1] for i in range(4)]
    attn_groups = [[0, 2, 4, 6], [1, 3, 5, 7]]

    with TileContext(nc) as tc:
        with (
            tc.tile_pool(name="const", bufs=1) as cpool,
            tc.tile_pool(name="sb", bufs=3) as sb,
            tc.tile_pool(name="sbg", bufs=2) as sbg,
            tc.tile_pool(name="sbeq", bufs=4) as sbeq,
            tc.tile_pool(name="psum", bufs=2, space="PSUM") as psum,
        ):
            def cload(src, shape, tag, dt=F32):
                t = cpool.tile(shape, dt, tag=tag)
                nc.sync.dma_start(out=t[:], in_=src[:, :])
                return t

            iota_t = cload(iota_in, [128, 128], "c_iota")
            ident_t = cload(ident_in, [128, 128], "c_ident", BF16)
            wm1_t = cload(wm1, [128, 128], "c_wm1", BF16)
            wr1_t = cload(wr1, [128, 128], "c_wr1", BF16)
            wm2_t = cload(wm2, [128, 128], "c_wm2", BF16)
            wr2_t = cload(wr2, [128, 128], "c_wr2", BF16)
            qs_t = cload(qs_rep, [128, 128], "c_qs", BF16)
            sel_t = cload(sel, [128, 4], "c_sel")
            score_sb = cpool.tile([128, ng2], F32, tag="c_score")

            pools = (sb, sbg, psum, sbeq)

            nc.sync.dma_start(out=e_loc[:, :], in_=e_sh[:, :])
            nc.gpsimd.collective_compute(
                "AllGather", mybir.AluOpType.bypass,
                replica_groups=all_group,
                ins=[e_loc[:, :]], outs=[e_full[:, :]])

            _emit_layer(nc, tc, pools, e_full, g1_lo, g1_hi, g1_dl, g1_dg,
                        g1_idxd, wm1_t, wr1_t, ng1, nb1, iota_t, ident_t,
                        x1_half, half)

            nc.gpsimd.collective_compute(
                "AllGather", mybir.AluOpType.bypass,
                replica_groups=pair_groups,
                ins=[x1_half[:, :]], outs=[x1_full[:, :]])

            def score_hook(g, xn):
                t = sb.tile([128, 128], F32, tag="sc_tmp")
                nc.vector.tensor_tensor(out=t[:], in0=xn, in1=qs_t[:],
                                        op=mybir.AluOpType.mult)
                nc.vector.reduce_sum(out=score_sb[:, g:g + 1], in_=t[:],
                                     axis=mybir.AxisListType.X)

            _emit_layer(nc, tc, pools, x1_full, g2_lo, g2_hi, g2_dl, g2_dg,
                        g2_idxd, wm2_t, wr2_t, ng2, nb2, iota_t, ident_t,
                        x2b, ng2 * 128, hook=score_hook)

            nc.sync.dma_start(out=sc_in[:, :].rearrange("t p -> p t"),
                              in_=score_sb[:, :])
            nc.gpsimd.collective_compute(
                "AllGather", mybir.AluOpType.bypass,
                replica_groups=attn_groups,
                ins=[sc_in[:, :]], outs=[sc_all[:, :]])

            # softmax over 4 metapaths (elementwise across four [128,ng2] tiles)
            s_t = []
            for p in range(4):
                st = cpool.tile([128, ng2], F32, tag=f"s{p}")
                nc.sync.dma_start(
                    out=st[:],
                    in_=sc_all[p * ng2:(p + 1) * ng2, :].rearrange("t p -> p t"))
                s_t.append(st)
            m = cpool.tile([128, ng2], F32, tag="c_m")
            nc.vector.tensor_tensor(out=m[:], in0=s_t[0][:], in1=s_t[1][:],
                                    op=mybir.AluOpType.max)
            for p in (2, 3):
                nc.vector.tensor_tensor(out=m[:], in0=m[:], in1=s_t[p][:],
                                        op=mybir.AluOpType.max)
            e_t = []
            for p in range(4):
                dt_ = cpool.tile([128, ng2], F32, tag=f"d{p}")
                nc.vector.tensor_tensor(out=dt_[:], in0=s_t[p][:], in1=m[:],
                                        op=mybir.AluOpType.subtract)
                et = cpool.tile([128, ng2], F32, tag=f"e{p}")
                nc.scalar.activation(out=et[:], in_=dt_[:],
                                     func=mybir.ActivationFunctionType.Exp)
                e_t.append(et)
            z = cpool.tile([128, ng2], F32, tag="c_z")
            nc.vector.tensor_tensor(out=z[:], in0=e_t[0][:], in1=e_t[1][:],
                                    op=mybir.AluOpType.add)
            for p in (2, 3):
                nc.vector.tensor_tensor(out=z[:], in0=z[:], in1=e_t[p][:],
                                        op=mybir.AluOpType.add)
            rz = cpool.tile([128, ng2], F32, tag="c_rz")
            nc.vector.reciprocal(out=rz[:], in_=z[:])
            wown = cpool.tile([128, ng2], F32, tag="c_wown")
            acc = cpool.tile([128, ng2], F32, tag="c_acc")
            nc.vector.tensor_scalar(out=wown[:], in0=e_t[0][:],
                                    scalar1=sel_t[:, 0:1], scalar2=None,
                                    op0=mybir.AluOpType.mult)
            for p in (1, 2, 3):
                nc.vector.tensor_scalar(out=acc[:], in0=e_t[p][:],
                                        scalar1=sel_t[:, p:p + 1], scalar2=None,
                                        op0=mybir.AluOpType.mult)
                nc.vector.tensor_tensor(out=wown[:], in0=wown[:], in1=acc[:],
                                        op=mybir.AluOpType.add)
            nc.vector.tensor_tensor(out=wown[:], in0=wown[:], in1=rz[:],
                                    op=mybir.AluOpType.mult)
            # weighted partials, batched BF groups per DMA
            for g0 in range(0, ng2, BF):
                bw = min(BF, ng2 - g0)
                xt = sb.tile([128, bw * 128], BF16, tag="attn_x")
                nc.sync.dma_start(
                    out=xt[:].rearrange("p (a f) -> p a f", f=128),
                    in_=x2b[g0 * 128:(g0 + bw) * 128, :]
                    .rearrange("(a t) f -> t a f", t=128))
                wt = sb.tile([128, bw * 128], BF16, tag="attn_w")
                for j in range(bw):
                    nc.vector.tensor_scalar(
                        out=wt[:, j * 128:(j + 1) * 128],
                        in0=xt[:, j * 128:(j + 1) * 128],
                        scalar1=wown[:, g0 + j:g0 + j + 1], scalar2=None,
                        op0=mybir.AluOpType.mult)
                nc.sync.dma_start(
                    out=rs_in[g0 * 128:(g0 + bw) * 128, :]
                    .rearrange("(a t) f -> t a f", t=128),
                    in_=wt[:].rearrange("p (a f) -> p a f", f=128))

            nc.gpsimd.collective_compute(
                "ReduceScatter", mybir.AluOpType.add,
                replica_groups=attn_groups,
                ins=[rs_in[:, :]], outs=[rs_out[:, :]])

            # rs_out [nrs,128] -> out_part, bounced through SBUF
            nblk = nrs // 128
            fin = cpool.tile([128, nblk * 128], BF16, tag="c_fin")
            nc.sync.dma_start(
                out=fin[:].rearrange("p (a f) -> p a f", f=128),
                in_=rs_out[:, :].rearrange("(a t) f -> t a f", t=128))
            nc.sync.dma_start(
                out=out_part[:, :].rearrange("(a t) f -> t a f", t=128),
                in_=fin[:].rearrange("p (a f) -> p a f", f=128))
    return nc


# ----------------------------------------------------------------- kernel()

def kernel(E, metapath_emb, W_root, W_rel, b, Wq, bq, edge_index, eids,
           nreg=50000, trace=False, debug=False):
    bf16 = _np_bf16()
    P = edge_index.shape[0]
    n = eids.shape[1]
    d = E.shape[1]
    etab = E.shape[0]
    scale = np.float32(1.0 / math.sqrt(d))
    assert P == 4 and d == 128 and n == 2 * nreg and nreg % 4 == 0
    assert etab % N_CORES == 0
    assert not np.any(np.asarray(b)), "nonzero bias not supported"

    E = np.asarray(E, np.float32)
    edge_index = np.asarray(edge_index)
    eids = np.asarray(eids).astype(np.int32)
    e_bf = E.astype(bf16)
    nsh = etab // N_CORES

    query = (np.asarray(metapath_emb, np.float32) @ np.asarray(Wq, np.float32)
             + np.asarray(bq, np.float32))
    query_scaled = query * scale

    ng1 = math.ceil(nreg / 128)
    ng2 = math.ceil((nreg // 2) / 128)

    # per-metapath: degree, dst-sorted edges
    metas = []
    for i in range(P):
        src = edge_index[i, 0].astype(np.int32)
        dst = edge_index[i, 1].astype(np.int32)
        deg = np.bincount(dst, minlength=n)
        assert deg.max() <= 255
        degc = np.maximum(deg, 1).astype(np.int32)
        order = np.argsort(dst, kind="stable")
        metas.append((degc, src[order], dst[order]))

    def rng(i, lo, hi):
        _, ssrc, sdst = metas[i]
        a, bb = np.searchsorted(sdst, [lo, hi])
        return ssrc[a:bb], sdst[a:bb]

    spans = []
    for c in range(N_CORES):
        i, h = c // 2, c % 2
        lo1, lo2 = h * nreg, h * (nreg // 2)
        spans.append((rng(i, lo1, lo1 + ng1 * 128),
                      rng(i, lo2, lo2 + ng2 * 128), lo1, lo2))

    nb1 = max(1, max(math.ceil(_group_max(s[0][1], s[2], ng1) / 128)
                     for s in spans))
    nb2 = max(1, max(math.ceil(_group_max(s[1][1], s[3], ng2) / 128)
                     for s in spans))

    iota = np.tile(np.arange(128, dtype=np.float32), (128, 1))
    ident = np.eye(128, dtype=bf16)

    in_maps = []
    for c in range(N_CORES):
        i, h = c // 2, c % 2
        (s1, d1), (s2, d2), lo1, lo2 = spans[c]
        degc = metas[i][0]
        # L1 gathers from the E table: compose indices through eids.
        lo16a, hi8a, dl8a, dg8a = _build_grids(eids[i][s1], d1, lo1, ng1, nb1,
                                               degc)
        # L2 gathers from x1_full: indices are node ids.
        lo16b, hi8b, dl8b, dg8b = _build_grids(s2, d2, lo2, ng2, nb2, degc)
        rows1 = np.minimum(lo1 + 128 * np.arange(ng1)[None, :]
                           + np.arange(128)[:, None], n - 1)
        idxd1 = eids[i][rows1].astype(np.int32)
        idxd2 = np.minimum(lo2 + 128 * np.arange(ng2)[None, :]
                           + np.arange(128)[:, None], n - 1).astype(np.int32)
        selm = np.zeros((128, 4), np.float32)
        selm[:, i] = 1.0
        in_maps.append(dict(
            e_sh=np.ascontiguousarray(e_bf[c * nsh:(c + 1) * nsh]),
            g1_lo=lo16a, g1_hi=hi8a, g1_dl=dl8a, g1_dg=dg8a,
            g1_idxd=np.ascontiguousarray(idxd1),
            g2_lo=lo16b, g2_hi=hi8b, g2_dl=dl8b, g2_dg=dg8b,
            g2_idxd=np.ascontiguousarray(idxd2),
            wm1=np.ascontiguousarray(W_rel[i, 0]).astype(bf16),
            wr1=np.ascontiguousarray(W_root[i, 0]).astype(bf16),
            wm2=np.ascontiguousarray(W_rel[i, 1]).astype(bf16),
            wr2=np.ascontiguousarray(W_root[i, 1]).astype(bf16),
            qs_rep=np.tile(query_scaled[i], (128, 1)).astype(bf16),
            sel=selm, iota=iota, ident=ident,
        ))

    nc = build_program(n, nreg, etab, ng1, nb1, ng2, nb2)
    nc.compile()
    kernel.last_nc = nc
    kernel.last_in_maps = in_maps
    res = run_bass_kernel_spmd(nc, in_maps, core_ids=list(range(N_CORES)),
                               trace=trace)

    q = nreg // 2
    a_rows = np.concatenate(
        [res.results[c]["out_part"].astype(np.float32) for c in (0, 2, 4, 6)],
        axis=0)[:q]
    b_rows = np.concatenate(
        [res.results[c]["out_part"].astype(np.float32) for c in (1, 3, 5, 7)],
        axis=0)[:q]
    out = np.concatenate([a_rows, b_rows], axis=0).astype(np.float32)
    kernel.last_results = res
    return out


# revision 25
# speedup vs baseline: 5.9395x; 2.9394x over previous
"""HAN layer (4 metapaths x 2-layer mean-RGCN + metapath attention) on 8 trn2 cores.

Sharding: cores (2i, 2i+1) handle metapath i. Within a pair, L1 splits dst into
halves [0,nreg)/[nreg,2*nreg); after an in-pair AllGather of x1, L2 splits the
NREG range into quarters. Attention: score AllGather + ReduceScatter over the 4
cores holding the same node range ({0,2,4,6} and {1,3,5,7}).

Device algorithm per layer (linearity: segment_sum(x[src]) @ Wm): edges are
host-sorted by dst into groups of 128 dsts; an indirect DMA gathers x[src] rows
for a group; per 128-edge chunk a selector eq[e,d] = (dl[e]==d)*rec[e] is built
on DVE and matmul-accumulated on PE into meanT = (segment_mean)^T in PSUM; two
dense matmuls + fused ReLU produce the group's 128 output rows, written
contiguously (no scatter anywhere).

Host->device transfer dominates the end-to-end time (narrow tunnel link with a
large per-buffer fixed cost), so inputs are shipped compressed and
consolidated into 6 buffers: E int8-quantized (dequant scale folded into the
layer-1 weights) and sharded 1/8 per core (device AllGather rebuilds the full
table; layer-1 gather indices are composed as eids[src] so the per-metapath x0
never ships), edge grids at 4 bytes/slot (u16 idx-lo + u8 [idx-hi<<6|deg] + u8
dst-local), unpacked on the DVE with is_ge/subtract (no mod). All device
compute is bf16 with f32 PSUM accumulation.
"""

import math
import numpy as np

try:
    # run_bass_kernel_spmd re-jits an identical XLA wrapper on every call;
    # the persistent compilation cache makes those re-jits near-free.
    import jax as _jax
    _jax.config.update("jax_compilation_cache_dir", "/tmp/jax_cc")
    _jax.config.update("jax_persistent_cache_min_entry_size_bytes", -1)
    _jax.config.update("jax_persistent_cache_min_compile_time_secs", 0.0)
except Exception:
    pass

import concourse.bass as bass
import concourse.bacc as bacc
import concourse.mybir as mybir
from concourse.tile import TileContext
from concourse.bass_utils import run_bass_kernel_spmd

F32 = mybir.dt.float32
BF16 = mybir.dt.bfloat16
I32 = mybir.dt.int32
I8 = mybir.dt.int8
U16 = mybir.dt.uint16
U8 = mybir.dt.uint8

N_CORES = 8
BF = 4     # output groups batched per store DMA
CH = 16    # groups per grid-load DMA


def _np_bf16():
    import ml_dtypes
    return ml_dtypes.bfloat16


# ----------------------------------------------------------------- host prep

def _build_grids(idxs, dsts, lo, ng, nb, deg):
    """Packed grid: grid[p, g*nb + b] = edge at (partition p, chunk b) of group
    g; the indirect-DMA flat order j = p*nb + b lands row j at out-partition p,
    column block b. Ships as u16 idx-low + u8 (idx-high<<6 | deg) + u8
    dst-local. Empty slots: dl=128 (selector row all-zero), pk=1 (finite
    reciprocal)."""
    g = (dsts - lo) >> 7
    starts = np.searchsorted(dsts, lo + 128 * np.arange(ng))
    slot = np.arange(len(dsts)) - starts[g]
    p = slot & 127
    b = slot >> 7
    col = g * nb + b
    S = nb * ng
    degv = deg[dsts]
    assert degv.size == 0 or degv.max() <= 63
    assert idxs.size == 0 or idxs.max() < (1 << 18)
    lo16 = np.zeros((128, S), np.uint16)
    pk8 = np.ones((128, S), np.uint8)
    dl8 = np.full((128, S), 128, np.uint8)
    lo16[p, col] = (idxs & 0xFFFF).astype(np.uint16)
    pk8[p, col] = (((idxs >> 16) << 6) | degv).astype(np.uint8)
    dl8[p, col] = (dsts - lo - (g << 7)).astype(np.uint8)
    return lo16, pk8, dl8


def _group_max(dsts, lo, ng):
    starts = np.searchsorted(dsts, lo + 128 * np.arange(ng + 1))
    return int(np.diff(starts).max()) if len(dsts) else 1


# ------------------------------------------------------------- device build

def _emit_layer(nc, tc, pools, table, table_i8, n_hi, glo, lo_off, gu8,
                pk_off, dl_off, gidxd, xd_off, wm_t, wr_t, ng, nb, iota_t,
                ident_t, out_dram, rows_total, hook=None):
    sb, sbg, psum, sbeq = pools
    stage = None
    for g in range(ng):
        if g % CH == 0:
            w = min(CH, ng - g)
            lot = sbg.tile([128, nb * w], U16, tag="lot")
            nc.sync.dma_start(
                out=lot[:], in_=glo[:, lo_off + g * nb:lo_off + (g + w) * nb])
            pkt = sbg.tile([128, nb * w], U8, tag="pkt")
            nc.sync.dma_start(
                out=pkt[:], in_=gu8[:, pk_off + g * nb:pk_off + (g + w) * nb])
            dlt8 = sbg.tile([128, nb * w], U8, tag="dlt8")
            nc.sync.dma_start(
                out=dlt8[:], in_=gu8[:, dl_off + g * nb:dl_off + (g + w) * nb])
            idxdt = sbg.tile([128, w], I32, tag="idxdt")
            nc.sync.dma_start(out=idxdt[:],
                              in_=gidxd[:, xd_off + g:xd_off + g + w])
            # unpack pk = hi<<6 | deg without mod: 64*hi via is_ge steps,
            # idx = lo + 65536*hi (exact in f32: < 2^24), rec = 1/deg
            pkf = sbg.tile([128, nb * w], F32, tag="pkf")
            nc.vector.tensor_copy(out=pkf[:], in_=pkt[:])
            hi64 = sbg.tile([128, nb * w], F32, tag="hi64")
            nc.vector.tensor_scalar(out=hi64[:], in0=pkf[:], scalar1=64.0,
                                    scalar2=64.0, op0=mybir.AluOpType.is_ge,
                                    op1=mybir.AluOpType.mult)
            for k in range(1, n_hi):
                hpart = sbg.tile([128, nb * w], F32, tag="hpart")
                nc.vector.tensor_scalar(
                    out=hpart[:], in0=pkf[:], scalar1=64.0 * (k + 1),
                    scalar2=64.0, op0=mybir.AluOpType.is_ge,
                    op1=mybir.AluOpType.mult)
                nc.vector.tensor_tensor(out=hi64[:], in0=hi64[:], in1=hpart[:],
                                        op=mybir.AluOpType.add)
            dgf = sbg.tile([128, nb * w], F32, tag="dgf")
            nc.vector.tensor_tensor(out=dgf[:], in0=pkf[:], in1=hi64[:],
                                    op=mybir.AluOpType.subtract)
            lof = sbg.tile([128, nb * w], F32, tag="lof")
            nc.vector.tensor_copy(out=lof[:], in_=lot[:])
            nc.vector.tensor_scalar(out=hi64[:], in0=hi64[:], scalar1=1024.0,
                                    scalar2=None, op0=mybir.AluOpType.mult)
            idxt = sbg.tile([128, nb * w], I32, tag="idxt")
            nc.vector.tensor_tensor(out=idxt[:], in0=hi64[:], in1=lof[:],
                                    op=mybir.AluOpType.add)
            dlf = sbg.tile([128, nb * w], F32, tag="dlf")
            nc.vector.tensor_copy(out=dlf[:], in_=dlt8[:])
            recf = sbg.tile([128, nb * w], F32, tag="recf")
            nc.vector.reciprocal(out=recf[:], in_=dgf[:])
        o = (g % CH) * nb

        if table_i8:
            msgs_raw = sb.tile([128, nb * 128], I8, tag="msgs_raw")
        else:
            msgs_raw = sb.tile([128, nb * 128], BF16, tag="msgs")
        for b in range(nb):
            nc.gpsimd.indirect_dma_start(
                out=msgs_raw[:, b * 128:(b + 1) * 128], out_offset=None,
                in_=table[:],
                in_offset=bass.IndirectOffsetOnAxis(
                    ap=idxt[:, o + b:o + b + 1], axis=0))
        if table_i8:
            msgs = sb.tile([128, nb * 128], BF16, tag="msgs")
            nc.vector.tensor_copy(out=msgs[:], in_=msgs_raw[:])
        else:
            msgs = msgs_raw

        meant_ps = psum.tile([128, 128], F32, space="PSUM", tag="meant")
        for b in range(nb):
            eq = sbeq.tile([128, 128], BF16, tag="eq")
            nc.vector.tensor_scalar(
                out=eq[:], in0=iota_t[:],
                scalar1=dlf[:, o + b:o + b + 1], scalar2=recf[:, o + b:o + b + 1],
                op0=mybir.AluOpType.is_equal, op1=mybir.AluOpType.mult)
            nc.tensor.matmul(out=meant_ps[:], lhsT=msgs[:, b * 128:(b + 1) * 128],
                             rhs=eq[:], start=(b == 0), stop=(b == nb - 1))
        meant = sb.tile([128, 128], BF16, tag="meant_sb")
        nc.vector.tensor_copy(out=meant[:], in_=meant_ps[:])

        if table_i8:
            xd_raw = sb.tile([128, 128], I8, tag="xd_raw")
        else:
            xd_raw = sb.tile([128, 128], BF16, tag="xd")
        nc.gpsimd.indirect_dma_start(
            out=xd_raw[:], out_offset=None, in_=table[:],
            in_offset=bass.IndirectOffsetOnAxis(
                ap=idxdt[:, g % CH:g % CH + 1], axis=0))
        if table_i8:
            xd = sb.tile([128, 128], BF16, tag="xd")
            nc.vector.tensor_copy(out=xd[:], in_=xd_raw[:])
        else:
            xd = xd_raw
        xdt_ps = psum.tile([128, 128], BF16, space="PSUM", tag="xdt")
        nc.tensor.transpose(out=xdt_ps[:], in_=xd[:], identity=ident_t[:])
        xdt = sb.tile([128, 128], BF16, tag="xdt_sb")
        nc.vector.tensor_copy(out=xdt[:], in_=xdt_ps[:])

        h_ps = psum.tile([128, 128], F32, space="PSUM", tag="hps")
        nc.tensor.matmul(out=h_ps[:], lhsT=meant[:], rhs=wm_t[:],
                         start=True, stop=False)
        nc.tensor.matmul(out=h_ps[:], lhsT=xdt[:], rhs=wr_t[:],
                         start=False, stop=True)

        gb = g % BF
        if gb == 0:
            bw = min(BF, ng - g)
            stage = sb.tile([128, bw * 128], BF16, tag="xn_stage")
        xn = stage[:, gb * 128:(gb + 1) * 128]
        nc.scalar.activation(out=xn, in_=h_ps[:],
                             func=mybir.ActivationFunctionType.Relu)
        if hook is not None:
            hook(g, xn)
        if gb == bw - 1:
            g0 = g - gb
            rows = min((gb + 1) * 128, rows_total - g0 * 128)
            nfull = rows // 128
            if nfull > 0:
                nc.sync.dma_start(
                    out=out_dram[g0 * 128:g0 * 128 + nfull * 128, :]
                    .rearrange("(a t) f -> t a f", t=128),
                    in_=stage[:, :nfull * 128]
                    .rearrange("p (a f) -> p a f", f=128))
            rem = rows - nfull * 128
            if rem > 0:
                nc.sync.dma_start(
                    out=out_dram[g0 * 128 + nfull * 128:
                                 g0 * 128 + nfull * 128 + rem, :],
                    in_=stage[:rem, nfull * 128:(nfull + 1) * 128])


def build_program(n, nreg, etab, ng1, nb1, ng2, nb2):
    nc = bacc.Bacc("TRN2", target_bir_lowering=False, debug=False,
                   num_devices=N_CORES)
    half = nreg
    nsh = etab // N_CORES
    nrs = (ng2 * 128) // 4  # ReduceScatter rows per rank
    S1, S2 = nb1 * ng1, nb2 * ng2

    def ei(name, shape, dt=F32):
        return nc.dram_tensor(name, shape, dt, kind="ExternalInput")

    # consolidated inputs (per-buffer transfer overhead is large)
    e_sh = ei("e_sh", [nsh, 128], I8)
    g_lo = ei("g_lo", [128, S1 + S2], U16)          # [g1_lo | g2_lo]
    g_u8 = ei("g_u8", [128, 2 * (S1 + S2)], U8)     # [g1_pk|g1_dl|g2_pk|g2_dl]
    g_xd = ei("g_xd", [128, ng1 + ng2], I32)        # [idxd1 | idxd2]
    cst = ei("cst", [128, 132])                     # [iota | sel]
    wts = ei("wts", [128, 6 * 128], BF16)  # [wm1|wr1|wm2|wr2|qs|ident]

    out_part = nc.dram_tensor("out_part", [nrs, 128], BF16,
                              kind="ExternalOutput")

    e_loc = nc.dram_tensor("e_loc", [nsh, 128], I8)
    e_full = nc.dram_tensor("e_full", [nsh * N_CORES, 128], I8)
    x1_half = nc.dram_tensor("x1_half", [half, 128], BF16)
    x1_full = nc.dram_tensor("x1_full", [n, 128], BF16)
    x2b = nc.dram_tensor("x2b", [ng2 * 128, 128], BF16)
    sc_in = nc.dram_tensor("sc_in", [ng2, 128], F32)
    sc_all = nc.dram_tensor("sc_all", [4 * ng2, 128], F32)
    rs_in = nc.dram_tensor("rs_in", [ng2 * 128, 128], BF16)
    rs_out = nc.dram_tensor("rs_out", [nrs, 128], BF16)

    all_group = [list(range(N_CORES))]
    pair_groups = [[2 * i, 2 * i + 1] for i in range(4)]
    attn_groups = [[0, 2, 4, 6], [1, 3, 5, 7]]

    with TileContext(nc) as tc:
        with (
            tc.tile_pool(name="const", bufs=1) as cpool,
            tc.tile_pool(name="sb", bufs=3) as sb,
            tc.tile_pool(name="sbg", bufs=2) as sbg,
            tc.tile_pool(name="sbeq", bufs=4) as sbeq,
            tc.tile_pool(name="psum", bufs=2, space="PSUM") as psum,
        ):
            def cload(src, c0, cols, tag, dt):
                t = cpool.tile([128, cols], dt, tag=tag)
                nc.sync.dma_start(out=t[:], in_=src[:, c0:c0 + cols])
                return t

            iota_t = cload(cst, 0, 128, "c_iota", F32)
            sel_t = cload(cst, 128, 4, "c_sel", F32)
            wm1_t = cload(wts, 0, 128, "c_wm1", BF16)
            wr1_t = cload(wts, 128, 128, "c_wr1", BF16)
            wm2_t = cload(wts, 256, 128, "c_wm2", BF16)
            wr2_t = cload(wts, 384, 128, "c_wr2", BF16)
            qs_t = cload(wts, 512, 128, "c_qs", BF16)
            ident_t = cload(wts, 640, 128, "c_ident", BF16)
            score_sb = cpool.tile([128, ng2], F32, tag="c_score")

            pools = (sb, sbg, psum, sbeq)

            nc.sync.dma_start(out=e_loc[:, :], in_=e_sh[:, :])
            nc.gpsimd.collective_compute(
                "AllGather", mybir.AluOpType.bypass,
                replica_groups=all_group,
                ins=[e_loc[:, :]], outs=[e_full[:, :]])

            _emit_layer(nc, tc, pools, e_full, True, 3, g_lo, 0, g_u8, 0, S1,
                        g_xd, 0, wm1_t, wr1_t, ng1, nb1, iota_t, ident_t,
                        x1_half, half)

            nc.gpsimd.collective_compute(
                "AllGather", mybir.AluOpType.bypass,
                replica_groups=pair_groups,
                ins=[x1_half[:, :]], outs=[x1_full[:, :]])

            def score_hook(g, xn):
                t = sb.tile([128, 128], F32, tag="sc_tmp")
                nc.vector.tensor_tensor(out=t[:], in0=xn, in1=qs_t[:],
                                        op=mybir.AluOpType.mult)
                nc.vector.reduce_sum(out=score_sb[:, g:g + 1], in_=t[:],
                                     axis=mybir.AxisListType.X)

            _emit_layer(nc, tc, pools, x1_full, False, 1, g_lo, S1, g_u8,
                        2 * S1, 2 * S1 + S2, g_xd, ng1, wm2_t, wr2_t, ng2, nb2,
                        iota_t, ident_t, x2b, ng2 * 128, hook=score_hook)

            nc.sync.dma_start(out=sc_in[:, :].rearrange("t p -> p t"),
                              in_=score_sb[:, :])
            nc.gpsimd.collective_compute(
                "AllGather", mybir.AluOpType.bypass,
                replica_groups=attn_groups,
                ins=[sc_in[:, :]], outs=[sc_all[:, :]])

            # softmax over 4 metapaths (elementwise across four [128,ng2] tiles)
            s_t = []
            for p in range(4):
                st = cpool.tile([128, ng2], F32, tag=f"s{p}")
                nc.sync.dma_start(
                    out=st[:],
                    in_=sc_all[p * ng2:(p + 1) * ng2, :].rearrange("t p -> p t"))
                s_t.append(st)
            m = cpool.tile([128, ng2], F32, tag="c_m")
            nc.vector.tensor_tensor(out=m[:], in0=s_t[0][:], in1=s_t[1][:],
                                    op=mybir.AluOpType.max)
            for p in (2, 3):
                nc.vector.tensor_tensor(out=m[:], in0=m[:], in1=s_t[p][:],
                                        op=mybir.AluOpType.max)
            e_t = []
            for p in range(4):
                dt_ = cpool.tile([128, ng2], F32, tag=f"d{p}")
                nc.vector.tensor_tensor(out=dt_[:], in0=s_t[p][:], in1=m[:],
                                        op=mybir.AluOpType.subtract)
                et = cpool.tile([128, ng2], F32, tag=f"e{p}")
                nc.scalar.activation(out=et[:], in_=dt_[:],
                                     func=mybir.ActivationFunctionType.Exp)
                e_t.append(et)
            z = cpool.tile([128, ng2], F32, tag="c_z")
            nc.vector.tensor_tensor(out=z[:], in0=e_t[0][:], in1=e_t[1][:],
                                    op=mybir.AluOpType.add)
            for p in (2, 3):
                nc.vector.tensor_tensor(out=z[:], in0=z[:], in1=e_t[p][:],
                                        op=mybir.AluOpType.add)
            rz = cpool.tile([128, ng2], F32, tag="c_rz")
            nc.vector.reciprocal(out=rz[:], in_=z[:])
            wown = cpool.tile([128, ng2], F32, tag="c_wown")
            acc = cpool.tile([128, ng2], F32, tag="c_acc")
            nc.vector.tensor_scalar(out=wown[:], in0=e_t[0][:],
                                    scalar1=sel_t[:, 0:1], scalar2=None,
                                    op0=mybir.AluOpType.mult)
            for p in (1, 2, 3):
                nc.vector.tensor_scalar(out=acc[:], in0=e_t[p][:],
                                        scalar1=sel_t[:, p:p + 1], scalar2=None,
                                        op0=mybir.AluOpType.mult)
                nc.vector.tensor_tensor(out=wown[:], in0=wown[:], in1=acc[:],
                                        op=mybir.AluOpType.add)
            nc.vector.tensor_tensor(out=wown[:], in0=wown[:], in1=rz[:],
                                    op=mybir.AluOpType.mult)

            # weighted partials, batched BF groups per DMA
            for g0 in range(0, ng2, BF):
                bw = min(BF, ng2 - g0)
                xt = sb.tile([128, bw * 128], BF16, tag="attn_x")
                nc.sync.dma_start(
                    out=xt[:].rearrange("p (a f) -> p a f", f=128),
                    in_=x2b[g0 * 128:(g0 + bw) * 128, :]
                    .rearrange("(a t) f -> t a f", t=128))
                wt = sb.tile([128, bw * 128], BF16, tag="attn_w")
                for j in range(bw):
                    nc.vector.tensor_scalar(
                        out=wt[:, j * 128:(j + 1) * 128],
                        in0=xt[:, j * 128:(j + 1) * 128],
                        scalar1=wown[:, g0 + j:g0 + j + 1], scalar2=None,
                        op0=mybir.AluOpType.mult)
                nc.sync.dma_start(
                    out=rs_in[g0 * 128:(g0 + bw) * 128, :]
                    .rearrange("(a t) f -> t a f", t=128),
                    in_=wt[:].rearrange("p (a f) -> p a f", f=128))

            nc.gpsimd.collective_compute(
                "ReduceScatter", mybir.AluOpType.add,
                replica_groups=attn_groups,
                ins=[rs_in[:, :]], outs=[rs_out[:, :]])

            # rs_out [nrs,128] -> out_part, bounced through SBUF
            nblk = nrs // 128
            fin = cpool.tile([128, nblk * 128], BF16, tag="c_fin")
            nc.sync.dma_start(
                out=fin[:].rearrange("p (a f) -> p a f", f=128),
                in_=rs_out[:, :].rearrange("(a t) f -> t a f", t=128))
            nc.sync.dma_start(
                out=out_part[:, :].rearrange("(a t) f -> t a f", t=128),
                in_=fin[:].rearrange("p (a f) -> p a f", f=128))
    return nc


# ----------------------------------------------------------------- kernel()

def kernel(E, metapath_emb, W_root, W_rel, b, Wq, bq, edge_index, eids,
           nreg=50000, trace=False, debug=False):
    bf16 = _np_bf16()
    P = edge_index.shape[0]
    n = eids.shape[1]
    d = E.shape[1]
    etab = E.shape[0]
    scale = np.float32(1.0 / math.sqrt(d))
    assert P == 4 and d == 128 and n == 2 * nreg and nreg % 4 == 0
    assert not np.any(np.asarray(b)), "nonzero bias not supported"

    E = np.asarray(E, np.float32)
    edge_index = np.asarray(edge_index)
    eids = np.asarray(eids).astype(np.int32)
    # keep only E rows some eids references, then int8-quantize (dequant
    # scale folds into the L1 weights)
    used = np.zeros(etab, bool)
    used[eids.ravel()] = True
    remap = np.cumsum(used, dtype=np.int64) - 1
    eids = remap[eids].astype(np.int32)
    e_used = E[used]
    nu = e_used.shape[0]
    nsh = (nu + N_CORES - 1) // N_CORES
    etab = nsh * N_CORES
    e_scale = np.float32(max(float(np.abs(e_used).max()), 1e-30) / 127.0)
    e_q = np.zeros((etab, 128), np.int8)
    e_q[:nu] = np.clip(np.rint(e_used / e_scale), -127, 127)

    query = (np.asarray(metapath_emb, np.float32) @ np.asarray(Wq, np.float32)
             + np.asarray(bq, np.float32))
    query_scaled = query * scale

    ng1 = math.ceil(nreg / 128)
    ng2 = math.ceil((nreg // 2) / 128)

    # per-metapath: degree, dst-sorted edges
    metas = []
    for i in range(P):
        src = edge_index[i, 0].astype(np.int32)
        dst = edge_index[i, 1].astype(np.int32)
        deg = np.bincount(dst, minlength=n)
        degc = np.maximum(deg, 1).astype(np.int32)
        order = np.argsort(dst, kind="stable")
        metas.append((degc, src[order], dst[order]))

    def rng(i, lo, hi):
        _, ssrc, sdst = metas[i]
        a, bb = np.searchsorted(sdst, [lo, hi])
        return ssrc[a:bb], sdst[a:bb]

    spans = []
    for c in range(N_CORES):
        i, h = c // 2, c % 2
        lo1, lo2 = h * nreg, h * (nreg // 2)
        spans.append((rng(i, lo1, lo1 + ng1 * 128),
                      rng(i, lo2, lo2 + ng2 * 128), lo1, lo2))

    nb1 = max(1, max(math.ceil(_group_max(s[0][1], s[2], ng1) / 128)
                     for s in spans))
    nb2 = max(1, max(math.ceil(_group_max(s[1][1], s[3], ng2) / 128)
                     for s in spans))

    iota = np.tile(np.arange(128, dtype=np.float32), (128, 1))
    ident = np.eye(128, dtype=np.float32)

    in_maps = []
    for c in range(N_CORES):
        i, h = c // 2, c % 2
        (s1, d1), (s2, d2), lo1, lo2 = spans[c]
        degc = metas[i][0]
        # L1 gathers from the E table: compose indices through eids.
        lo16a, pk8a, dl8a = _build_grids(eids[i][s1], d1, lo1, ng1, nb1, degc)
        # L2 gathers from x1_full: indices are node ids.
        lo16b, pk8b, dl8b = _build_grids(s2, d2, lo2, ng2, nb2, degc)
        rows1 = np.minimum(lo1 + 128 * np.arange(ng1)[None, :]
                           + np.arange(128)[:, None], n - 1)
        idxd1 = eids[i][rows1].astype(np.int32)
        idxd2 = np.minimum(lo2 + 128 * np.arange(ng2)[None, :]
                           + np.arange(128)[:, None], n - 1).astype(np.int32)
        selm = np.zeros((128, 4), np.float32)
        selm[:, i] = 1.0
        wmat = np.concatenate([
            (np.asarray(W_rel[i, 0], np.float32) * e_scale),
            (np.asarray(W_root[i, 0], np.float32) * e_scale),
            np.asarray(W_rel[i, 1], np.float32),
            np.asarray(W_root[i, 1], np.float32),
            np.tile(query_scaled[i], (128, 1)).astype(np.float32),
            ident,
        ], axis=1).astype(bf16)
        in_maps.append(dict(
            e_sh=np.ascontiguousarray(e_q[c * nsh:(c + 1) * nsh]),
            g_lo=np.concatenate([lo16a, lo16b], axis=1),
            g_u8=np.concatenate([pk8a, dl8a, pk8b, dl8b], axis=1),
            g_xd=np.concatenate([idxd1, idxd2], axis=1).astype(np.int32),
            cst=np.concatenate([iota, selm], axis=1).astype(np.float32),
            wts=wmat,
        ))

    nc = build_program(n, nreg, etab, ng1, nb1, ng2, nb2)
    nc.compile()
    kernel.last_nc = nc
    kernel.last_in_maps = in_maps
    res = run_bass_kernel_spmd(nc, in_maps, core_ids=list(range(N_CORES)),
                               trace=trace)

    q = nreg // 2
    a_rows = np.concatenate(
        [res.results[c]["out_part"].astype(np.float32) for c in (0, 2, 4, 6)],
        axis=0)[:q]
    b_rows = np.concatenate(
        [res.results[c]["out_part"].astype(np.float32) for c in (1, 3, 5, 7)],
        axis=0)[:q]
    out = np.concatenate([a_rows, b_rows], axis=0).astype(np.float32)
    kernel.last_results = res
    return out
